# revision 37
# baseline (speedup 1.0000x reference)
"""AttentionPooling (segment softmax pooling) Trainium2 kernel.

Math (per reference):
    h = tanh(x @ W1 + b1); s = h @ W2 + b2
    w = softmax(s) within each contiguous segment (batch is sorted)
    out[b] = sum_{r in b} w_r * x[r]

Device algorithm (per core, segments sharded 512/core):
  Softmax is shift-invariant and |s| <= ||W2||_1 + |b2| ~ 9, so we skip the
  per-segment max and use e_r = exp(s_r + b2) directly (safe in fp32).
  out[b] = (sum e_r x_r) / (sum e_r): both sums come from one-hot matmuls
  contracted over rows, accumulated in PSUM over a 64-segment window shared
  by a SUP_GROUPS-group superblock, then scatter-accumulated (indirect DMA,
  compute_op=add) into a DRAM scratch [segs, 257] (256 pooled cols + 1 sum
  col); a pipelined end pass divides.

  Scores need x^T (D on partitions): host supplies x^T in bf16 (score path
  only shapes softmax weights; bf16 there perturbs the output by ~1e-3
  relative). Pooling reads x in natural layout (dtype configurable).

  Scheduling notes (from perfetto traces; see transcript):
  - The kernel is DMA-bound: ~69MB/core (x read twice, bf16, two layouts)
    at ~330GB/s effective => ~210us of DMA-active, total ~235-240us.
  - Scores are software-pipelined one group ahead of pooling so the esel
    (one-hot) vector ops never gate the pooling matmul chain.
  - Every DMA trigger parked on backpressure head-of-line-blocks its whole
    engine queue, so queues are segregated: sync=xT loads + out stores,
    gpsimd=x_nat loads + scatter + scratch loads, scalar=tanh/exp/flush
    (DMA-free), vector=esel only.
  - gpsimd tensor_scalar is ~5x slower than vector and its DMA_INDIRECT
    costs ~1.7us, so esel lives on vector and scatters are batched per
    superblock.

The program is identical across cores (SPMD); all data-dependent segment
offsets travel through input tensors (batch_local window ids + scatter row
indices), never through baked constants.
"""

import os
from contextlib import ExitStack

import numpy as np
import ml_dtypes

LAST_EXEC_NS = None

import concourse.bass as bass
import concourse.bacc as bacc
import concourse.tile as tile
from concourse import mybir
from concourse.bass import IndirectOffsetOnAxis
from concourse.bass_utils import run_bass_kernel_spmd

# ---- problem constants (hardcoded per contract) ----
N_TOTAL = 500000
D = 256
H = 128
NUM_SEGMENTS = 4096
N_CORES = 8
SEGS_PER_CORE = NUM_SEGMENTS // N_CORES  # 512

G_ROWS = 2048          # rows per group
TILES_PER_G = 16       # 128-row tiles per group
SUB_PER_G = 4          # 512-row subtiles per group (score matmuls)
W_SEG = 64             # segment window width per superblock (host asserts fit)
SUP_GROUPS = 3         # groups per scatter superblock (combined span < W_SEG)
SCRATCH_ROWS = 640     # 512 real segs + 128 pad rows for window overflow
PAD_BL = 255.0         # batch_local value for padding rows (never matches iota)

F32 = mybir.dt.float32
F32R = mybir.dt.float32r
BF16 = mybir.dt.bfloat16
I32 = mybir.dt.int32

# pooling input dtype: "f32" (safe) or "bf16" (halves pooling-read traffic)
X_POOL_DTYPE = "bf16"


def build_nc(n_groups: int, b2_val: float) -> bass.Bass:
    r_pad = n_groups * G_ROWS
    n_tiles = n_groups * TILES_PER_G
    xdt = F32 if X_POOL_DTYPE == "f32" else BF16

    nc = bacc.Bacc("TRN2", target_bir_lowering=False, debug=False)

    # DRAM I/O
    # esel/ones must match x's matmul dtype: f32r with f32 x, bf16 with bf16 x
    edt = F32R if xdt == F32 else BF16
    # x_nat carries D cols of x, a ones column (col 256, folds the seg_sum
    # matmul into the pooling matmul), and a zero pad col. Layout is
    # partition-major [128, n_tiles, 258]: x_nat[p, t, :] = x[128t + p, :],
    # so one group's load is a single contiguous 8.2KB run per partition.
    x_nat = nc.dram_tensor("x_nat", [128, n_tiles, D + 2], xdt, kind="ExternalInput")
    xT = nc.dram_tensor("xT", [D, r_pad], BF16, kind="ExternalInput")
    w1c = nc.dram_tensor("w1c", [2, 128, H], BF16, kind="ExternalInput")
    w2col = nc.dram_tensor("w2col", [H, 1], BF16, kind="ExternalInput")
    b1col = nc.dram_tensor("b1col", [H, 1], F32, kind="ExternalInput")
    iota64 = nc.dram_tensor("iota64", [128, W_SEG], BF16, kind="ExternalInput")
    bl_all = nc.dram_tensor("bl_all", [128, n_tiles], F32, kind="ExternalInput")
    n_sup = (n_groups + SUP_GROUPS - 1) // SUP_GROUPS
    seg_idx = nc.dram_tensor("seg_idx", [W_SEG, n_sup], I32, kind="ExternalInput")
    # ExternalOutput buffers are zero-initialized by the runtime — scratch
    # relies on that for its scatter-accumulate
    scratch = nc.dram_tensor("scratch", [SCRATCH_ROWS, 257], F32, kind="ExternalOutput")
    out = nc.dram_tensor("out", [SCRATCH_ROWS, D], F32, kind="ExternalOutput")

    with tile.TileContext(nc) as tc, ExitStack() as ctx:
        const_pool = ctx.enter_context(tc.tile_pool(name="const", bufs=1))
        xT_pool = ctx.enter_context(tc.tile_pool(name="xT", bufs=12))
        xnat_pool = ctx.enter_context(tc.tile_pool(name="xnat", bufs=8))
        h_pool = ctx.enter_context(tc.tile_pool(name="h", bufs=6))
        e_pool = ctx.enter_context(tc.tile_pool(name="e", bufs=4))
        esel_pool = ctx.enter_context(tc.tile_pool(name="esel", bufs=24))
        flush_pool = ctx.enter_context(tc.tile_pool(name="flush", bufs=8))
        fin_pool = ctx.enter_context(tc.tile_pool(name="fin", bufs=12))
        u_psum = ctx.enter_context(tc.tile_pool(name="u_ps", bufs=2, space="PSUM"))
        s_psum = ctx.enter_context(tc.tile_pool(name="s_ps", bufs=2, space="PSUM"))
        p_psum = ctx.enter_context(tc.tile_pool(name="p_ps", bufs=2, space="PSUM"))

        # ---- constants (scalar queue: sync stays free for the xt(0) load,
        # gpsimd for xn(0); scalar's first compute comes much later) ----
        w1c_t = const_pool.tile([128, 2 * H], BF16, tag="w1c")
        nc.scalar.dma_start(w1c_t[:, 0:H], w1c[0])
        nc.scalar.dma_start(w1c_t[:, H : 2 * H], w1c[1])
        w2_t = const_pool.tile([H, 1], BF16, tag="w2")
        nc.scalar.dma_start(w2_t[:], w2col[:, :])
        b1_t = const_pool.tile([H, 1], F32, tag="b1")
        nc.scalar.dma_start(b1_t[:], b1col[:, :])
        iota_t = const_pool.tile([128, W_SEG], BF16, tag="iota")
        nc.scalar.dma_start(iota_t[:], iota64[:, :])
        bl_t = const_pool.tile([128, n_tiles], F32, tag="bl")
        nc.scalar.dma_start(bl_t[:], bl_all[:, :])
        sidx_t = const_pool.tile([W_SEG, n_sup], I32, tag="sidx")
        nc.scalar.dma_start(sidx_t[:], seg_idx[:, :])

        # ---- main loop over row groups, scores pipelined one group ahead ----
        # PE queue order per iteration: scores(g+1) then pooling(g). While PE
        # runs scores(g+1), vector builds esel(g) from the already-finished
        # e(g), so pooling(g)'s matmuls never wait tile-by-tile on vector.
        #
        # DMA trigger queues are segregated so that a trigger parked on DMA
        # backpressure never head-of-line-blocks latency-critical compute:
        # sync carries the xt loads, gpsimd the xn loads (prefetched three
        # groups ahead of use) + scatter + normalize loads, scalar stays
        # DMA-free for tanh/exp/flush-copy, vector stays esel-only.
        e_tiles: dict[int, object] = {}
        xn_tiles: dict[int, object] = {}

        def emit_scores(g):
            # s_nat[p, c] = score(row 2048g + 128c + p); two PSUM tiles
            # (banks) so each half's exp can fire without waiting for (or
            # bank-serializing against) the other half's matmuls
            xt0 = xT_pool.tile([128, G_ROWS], BF16, tag="xt0")
            xt1 = xT_pool.tile([128, G_ROWS], BF16, tag="xt1")
            if g == 0:
                for q in range(4):
                    sl4 = slice(512 * q, 512 * (q + 1))
                    nc.sync.dma_start(xt0[:, sl4], xT[0:128, sl4])
                    nc.sync.dma_start(xt1[:, sl4], xT[128:256, sl4])
            else:
                nc.sync.dma_start(xt0[:], xT[0:128, g * G_ROWS : (g + 1) * G_ROWS])
                nc.sync.dma_start(xt1[:], xT[128:256, g * G_ROWS : (g + 1) * G_ROWS])
            e_t = e_pool.tile([128, TILES_PER_G], F32, tag="e")
            for half in range(2):
                snat = s_psum.tile([128, 8], F32, tag=f"snat{half}")
                for ii in range(SUB_PER_G // 2):
                    i = 2 * half + ii
                    sl = slice(512 * i, 512 * (i + 1))
                    u = u_psum.tile([H, 512], F32, tag="u")
                    nc.tensor.matmul(u[:], w1c_t[:, 0:H], xt0[:, sl], start=True, stop=False)
                    nc.tensor.matmul(u[:], w1c_t[:, H : 2 * H], xt1[:, sl], start=False, stop=True)
                    h_t = h_pool.tile([H, 512], BF16, tag="h")
                    nc.scalar.activation(h_t[:], u[:], mybir.ActivationFunctionType.Tanh, bias=b1_t[:, 0:1])
                    for j in range(4):
                        lc = 4 * ii + j
                        nc.tensor.matmul(
                            snat[:, lc : lc + 1],
                            h_t[:, 128 * j : 128 * (j + 1)],
                            w2_t[:],
                            start=(lc == 0),
                            stop=(lc == 7),
                            skip_group_check=True,
                        )
                nc.scalar.activation(
                    e_t[:, 8 * half : 8 * (half + 1)],
                    snat[:],
                    mybir.ActivationFunctionType.Exp,
                    bias=float(b2_val),
                )
            e_tiles[g] = e_t

        def emit_xn_load(g):
            xn = xnat_pool.tile([128, TILES_PER_G * (D + 2)], xdt, tag="xn")
            t0 = g * TILES_PER_G
            xn3 = xn[:].rearrange("p (t d) -> p t d", d=D + 2)
            if g < 2:
                # warmup: 4-tile slabs so pooling(0) starts on the first
                # quarter instead of the whole 1MB group load
                for q in range(4):
                    nc.gpsimd.dma_start(
                        xn3[:, 4 * q : 4 * (q + 1), :],
                        x_nat[:, t0 + 4 * q : t0 + 4 * (q + 1), :],
                    )
            else:
                nc.gpsimd.dma_start(xn3, x_nat[:, t0 : t0 + TILES_PER_G, :])
            xn_tiles[g] = xn

        # pooling accumulates [64 segs, 256 pooled + 1 sum] in ONE PSUM tile
        # across a superblock of SUP_GROUPS groups (their combined segment
        # span fits the 64-wide window; host asserts), so the flush copy +
        # scatter-accumulate run once per superblock instead of per group —
        # 3x fewer gpsimd scatter instructions and DMA-semaphore recycles.
        sup_state: dict[str, object] = {"pooled": None}

        def emit_pooling(g):
            e_t = e_tiles.pop(g)
            xn = xn_tiles.pop(g)
            sup = g // SUP_GROUPS
            g_in_sup = g % SUP_GROUPS
            is_last_in_sup = g_in_sup == SUP_GROUPS - 1 or g == n_groups - 1
            if g_in_sup == 0:
                pooled_new = p_psum.tile([128, 257], F32, tag="pooled")
                sup_state["pooled"] = pooled_new
            pooled = sup_state["pooled"]
            for c in range(TILES_PER_G):
                t_abs = g * TILES_PER_G + c
                esel = esel_pool.tile([128, W_SEG], edt, tag="esel")
                nc.vector.tensor_scalar(
                    esel[:],
                    iota_t[:],
                    bl_t[:, t_abs : t_abs + 1],
                    e_t[:, c : c + 1],
                    mybir.AluOpType.is_equal,
                    mybir.AluOpType.mult,
                )
                rhs = xn[:, c * (D + 2) : c * (D + 2) + 257]
                if xdt == F32:
                    rhs = rhs.bitcast(F32R)
                nc.tensor.matmul(
                    pooled[0:W_SEG, 0:257], esel[:], rhs,
                    start=(g_in_sup == 0 and c == 0),
                    stop=(is_last_in_sup and c == TILES_PER_G - 1),
                    skip_group_check=True,
                )
            if is_last_in_sup:
                # flush: psum -> sbuf (scalar engine) -> scatter-accumulate
                fl = flush_pool.tile([W_SEG, 257], F32, tag="fl")
                nc.scalar.activation(fl[:], pooled[0:W_SEG, :], mybir.ActivationFunctionType.Copy)
                nc.gpsimd.indirect_dma_start(
                    scratch[:, :],
                    IndirectOffsetOnAxis(ap=sidx_t[:, sup : sup + 1], axis=0),
                    fl[:],
                    None,
                    compute_op=mybir.AluOpType.add,
                )

        emit_scores(0)
        for gp in range(min(3, n_groups)):
            emit_xn_load(gp)
        for g in range(n_groups):
            if g + 1 < n_groups:
                emit_scores(g + 1)
            if g + 3 < n_groups:
                emit_xn_load(g + 3)
            emit_pooling(g)

        # ---- final normalize, fully pipelined at the end: all block loads
        # issue together (the first carries the scatter fan-in wait), then
        # the vector divides, then the stores. Mid-stream normalize was tried
        # and reverted: its scatter-fan-in waits head-of-line-blocked the
        # esel stream (vector) or the xt prefetch (sync) for ~7us a block.
        n_blocks = SEGS_PER_CORE // 128  # 4 real blocks; pad rows never read
        ft_tiles = []
        for b in range(n_blocks):
            ft = fin_pool.tile([128, 257], F32, tag="ft")
            nc.gpsimd.dma_start(ft[:], scratch[128 * b : 128 * (b + 1), :])
            ft_tiles.append(ft)
        for b in range(n_blocks):
            ft = ft_tiles[b]
            rec = fin_pool.tile([128, 1], F32, tag="rec")
            nc.vector.reciprocal(rec[:], ft[:, D : D + 1])
            ot = fin_pool.tile([128, D], F32, tag="ot")
            nc.vector.tensor_scalar(
                ot[:], ft[:, 0:D], rec[:, 0:1], None, mybir.AluOpType.mult,
            )
            nc.sync.dma_start(out[128 * b : 128 * (b + 1), :], ot[:])

    return nc


def kernel(x, batch, W1, b1, W2, b2):
    x = np.asarray(x, dtype=np.float32)
    batch = np.asarray(batch)
    W1 = np.asarray(W1, dtype=np.float32)
    b1 = np.asarray(b1, dtype=np.float32)
    W2 = np.asarray(W2, dtype=np.float32)
    b2 = np.asarray(b2, dtype=np.float32)
    n, d = x.shape
    assert d == D

    bounds = np.searchsorted(batch, np.arange(NUM_SEGMENTS + 1))
    core_starts = [int(bounds[SEGS_PER_CORE * m]) for m in range(N_CORES + 1)]
    rows_per_core = [core_starts[m + 1] - core_starts[m] for m in range(N_CORES)]
    n_groups = max(1, int(np.ceil(max(rows_per_core) / G_ROWS)))
    r_pad = n_groups * G_ROWS
    n_tiles = n_groups * TILES_PER_G

    xdt = np.float32 if X_POOL_DTYPE == "f32" else ml_dtypes.bfloat16

    # shared constant inputs
    w1c = np.ascontiguousarray(
        W1.reshape(2, 128, H).astype(ml_dtypes.bfloat16)
    )
    w2col = np.ascontiguousarray(W2.reshape(H, 1).astype(ml_dtypes.bfloat16))
    b1col = np.ascontiguousarray(b1.reshape(H, 1))
    iota64 = np.broadcast_to(np.arange(W_SEG), (128, W_SEG)).astype(ml_dtypes.bfloat16)
    b2_val = float(b2.reshape(-1)[0])

    in_maps = []
    core_s0s = []
    for m in range(N_CORES):
        rs, re = core_starts[m], core_starts[m + 1]
        rows = re - rs
        xm = x[rs:re]
        x_flat = np.zeros((r_pad, D + 2), dtype=xdt)
        x_flat[:rows, :D] = xm.astype(xdt)
        x_flat[:rows, D] = xdt(1.0)
        # partition-major: x_nat[p, t, :] = x_flat[128t + p, :]
        x_nat = np.ascontiguousarray(
            x_flat.reshape(n_tiles, 128, D + 2).transpose(1, 0, 2)
        )
        xT = np.zeros((D, r_pad), dtype=ml_dtypes.bfloat16)
        xT[:, :rows] = xm.T.astype(ml_dtypes.bfloat16)

        seg_local = (batch[rs:re] - SEGS_PER_CORE * m).astype(np.int64)
        assert seg_local.min() >= 0 and seg_local.max() < SEGS_PER_CORE

        bl = np.full((128, n_tiles), PAD_BL, dtype=np.float32)
        n_sup = (n_groups + SUP_GROUPS - 1) // SUP_GROUPS
        sidx = np.empty((W_SEG, n_sup), dtype=np.int32)
        s0s = np.empty(n_sup, dtype=np.int64)
        for sup in range(n_sup):
            lo = sup * SUP_GROUPS * G_ROWS
            hi = min((sup + 1) * SUP_GROUPS * G_ROWS, rows)
            if lo >= rows:
                s0 = SEGS_PER_CORE  # pad region
            else:
                s0 = int(seg_local[lo])
                span = int(seg_local[hi - 1]) - s0
                assert span < W_SEG, f"superblock seg span {span} >= {W_SEG}"
                rr = np.arange(lo, hi)
                p = rr % 128
                c = (rr % G_ROWS) // 128
                gg = rr // G_ROWS
                bl[p, gg * TILES_PER_G + c] = (seg_local[lo:hi] - s0).astype(np.float32)
            sidx[:, sup] = s0 + np.arange(W_SEG)
            s0s[sup] = s0
        core_s0s.append(s0s)
        in_maps.append(
            {
                "x_nat": x_nat,
                "xT": xT,
                "w1c": w1c,
                "w2col": w2col,
                "b1col": b1col,
                "iota64": iota64,
                "bl_all": bl,
                "seg_idx": sidx,
            }
        )

    nc = build_nc(n_groups, b2_val)
    if not nc.is_finalized():
        nc.finalize()
    trace = os.environ.get("KERNEL_TRACE", "0") == "1"
    kw = {}
    if trace:
        kw = dict(trace=True, tmpdir=os.environ.get("KERNEL_TRACE_DIR") or None)
    res = run_bass_kernel_spmd(nc, in_maps, core_ids=list(range(N_CORES)), **kw)
    global LAST_EXEC_NS
    LAST_EXEC_NS = res.exec_time_ns
    if trace:
        print(
            f"exec_time_ns={res.exec_time_ns} mean={res.mean_exec_time_ns} "
            f"max_core={res.max_exec_time_core_id}",
            flush=True,
        )
    outs = res.results

    full = np.empty((NUM_SEGMENTS, D), dtype=np.float32)
    for m in range(N_CORES):
        full[SEGS_PER_CORE * m : SEGS_PER_CORE * (m + 1)] = outs[m]["out"][
            :SEGS_PER_CORE
        ]
    return full



# revision 52
# speedup vs baseline: 1.0384x; 1.0384x over previous
"""AttentionPooling (segment softmax pooling) Trainium2 kernel.

Math (per reference):
    h = tanh(x @ W1 + b1); s = h @ W2 + b2
    w = softmax(s) within each contiguous segment (batch is sorted)
    out[b] = sum_{r in b} w_r * x[r]

Device algorithm (per core, segments sharded 512/core):
  Softmax is shift-invariant and |s| <= ||W2||_1 + |b2| ~ 9, so we skip the
  per-segment max and use e_r = exp(s_r + b2) directly (safe in fp32).
  out[b] = (sum e_r x_r) / (sum e_r): both sums come from one-hot matmuls
  contracted over rows, accumulated in PSUM over a 64-segment window shared
  by a SUP_GROUPS-group superblock, then scatter-accumulated (indirect DMA,
  compute_op=add) into a DRAM scratch [segs, 257] (256 pooled cols + 1 sum
  col); a pipelined end pass divides.

  Scores need x^T (D on partitions): host supplies x^T in bf16 (score path
  only shapes softmax weights; bf16 there perturbs the output by ~1e-3
  relative). Pooling reads x in natural layout (dtype configurable).

  Scheduling notes (from perfetto traces; see transcript):
  - The kernel is DMA-bound: ~69MB/core (x read twice, bf16, two layouts)
    at ~330GB/s effective => ~210us of DMA-active, total ~235-240us.
  - Scores are software-pipelined one group ahead of pooling so the esel
    (one-hot) vector ops never gate the pooling matmul chain.
  - Every DMA trigger parked on backpressure head-of-line-blocks its whole
    engine queue, so queues are segregated: sync=xT loads + out stores,
    gpsimd=x_nat loads + scatter + scratch loads, scalar=tanh/exp/flush
    (DMA-free), vector=esel only.
  - gpsimd tensor_scalar is ~5x slower than vector and its DMA_INDIRECT
    costs ~1.7us, so esel lives on vector and scatters are batched per
    superblock.

The program is identical across cores (SPMD); all data-dependent segment
offsets travel through input tensors (batch_local window ids + scatter row
indices), never through baked constants.
"""

import os
from contextlib import ExitStack

import numpy as np
import ml_dtypes

LAST_EXEC_NS = None

import concourse.bass as bass
import concourse.bacc as bacc
import concourse.tile as tile
from concourse import mybir
from concourse.bass import IndirectOffsetOnAxis
from concourse.bass_utils import run_bass_kernel_spmd

# ---- problem constants (hardcoded per contract) ----
N_TOTAL = 500000
D = 256
H = 128
NUM_SEGMENTS = 4096
N_CORES = 8
SEGS_PER_CORE = NUM_SEGMENTS // N_CORES  # 512

G_ROWS = 2048          # rows per group
TILES_PER_G = 16       # 128-row tiles per group
SUB_PER_G = 4          # 512-row subtiles per group (score matmuls)
W_SEG = 64             # segment window width per superblock (host asserts fit)
SUP_GROUPS = 3         # groups per scatter superblock (combined span < W_SEG)
SCRATCH_ROWS = 640     # 512 real segs + 128 pad rows for window overflow
PAD_BL = 255.0         # batch_local value for padding rows (never matches iota)

F32 = mybir.dt.float32
F32R = mybir.dt.float32r
BF16 = mybir.dt.bfloat16
I32 = mybir.dt.int32

# pooling input dtype: "f32" (safe) or "bf16" (halves pooling-read traffic)
X_POOL_DTYPE = "bf16"


def build_nc(n_groups: int, b2_val: float) -> bass.Bass:
    r_pad = n_groups * G_ROWS
    n_tiles = n_groups * TILES_PER_G
    xdt = F32 if X_POOL_DTYPE == "f32" else BF16

    nc = bacc.Bacc("TRN2", target_bir_lowering=False, debug=False)

    # DRAM I/O
    # esel/ones must match x's matmul dtype: f32r with f32 x, bf16 with bf16 x
    edt = F32R if xdt == F32 else BF16
    # x_nat carries D cols of x, a ones column (col 256, folds the seg_sum
    # matmul into the pooling matmul), and a zero pad col. Layout is
    # partition-major [128, n_tiles, 258]: x_nat[p, t, :] = x[128t + p, :],
    # so one group's load is a single contiguous 8.2KB run per partition.
    x_nat = nc.dram_tensor("x_nat", [128, n_tiles, D + 2], xdt, kind="ExternalInput")
    xT = nc.dram_tensor("xT", [D, r_pad], BF16, kind="ExternalInput")
    w1c = nc.dram_tensor("w1c", [2, 128, H], BF16, kind="ExternalInput")
    w2col = nc.dram_tensor("w2col", [H, 1], BF16, kind="ExternalInput")
    b1col = nc.dram_tensor("b1col", [H, 1], F32, kind="ExternalInput")
    iota64 = nc.dram_tensor("iota64", [128, W_SEG], BF16, kind="ExternalInput")
    bl_all = nc.dram_tensor("bl_all", [128, n_tiles], F32, kind="ExternalInput")
    n_sup = (n_groups + SUP_GROUPS - 1) // SUP_GROUPS
    seg_idx = nc.dram_tensor("seg_idx", [W_SEG, n_sup], I32, kind="ExternalInput")
    # ExternalOutput buffers are zero-initialized by the runtime — scratch
    # relies on that for its scatter-accumulate
    scratch = nc.dram_tensor("scratch", [SCRATCH_ROWS, 257], F32, kind="ExternalOutput")
    out = nc.dram_tensor("out", [SCRATCH_ROWS, D], F32, kind="ExternalOutput")

    with tile.TileContext(nc) as tc, ExitStack() as ctx:
        const_pool = ctx.enter_context(tc.tile_pool(name="const", bufs=1))
        xT_pool = ctx.enter_context(tc.tile_pool(name="xT", bufs=12))
        xnat_pool = ctx.enter_context(tc.tile_pool(name="xnat", bufs=8))
        h_pool = ctx.enter_context(tc.tile_pool(name="h", bufs=6))
        e_pool = ctx.enter_context(tc.tile_pool(name="e", bufs=4))
        esel_pool = ctx.enter_context(tc.tile_pool(name="esel", bufs=24))
        flush_pool = ctx.enter_context(tc.tile_pool(name="flush", bufs=8))
        fin_pool = ctx.enter_context(tc.tile_pool(name="fin", bufs=3))
        u_psum = ctx.enter_context(tc.tile_pool(name="u_ps", bufs=2, space="PSUM"))
        s_psum = ctx.enter_context(tc.tile_pool(name="s_ps", bufs=2, space="PSUM"))
        p_psum = ctx.enter_context(tc.tile_pool(name="p_ps", bufs=2, space="PSUM"))

        # ---- constants (scalar queue: sync stays free for the xt(0) load,
        # gpsimd for xn(0); scalar's first compute comes much later) ----
        w1c_t = const_pool.tile([128, 2 * H], BF16, tag="w1c")
        nc.scalar.dma_start(w1c_t[:, 0:H], w1c[0])
        nc.scalar.dma_start(w1c_t[:, H : 2 * H], w1c[1])
        w2_t = const_pool.tile([H, 1], BF16, tag="w2")
        nc.scalar.dma_start(w2_t[:], w2col[:, :])
        b1_t = const_pool.tile([H, 1], F32, tag="b1")
        nc.scalar.dma_start(b1_t[:], b1col[:, :])
        iota_t = const_pool.tile([128, W_SEG], BF16, tag="iota")
        nc.scalar.dma_start(iota_t[:], iota64[:, :])
        bl_t = const_pool.tile([128, n_tiles], F32, tag="bl")
        nc.scalar.dma_start(bl_t[:], bl_all[:, :])
        sidx_t = const_pool.tile([W_SEG, n_sup], I32, tag="sidx")
        nc.scalar.dma_start(sidx_t[:], seg_idx[:, :])

        # ---- main loop over row groups, scores pipelined one group ahead ----
        # PE queue order per iteration: scores(g+1) then pooling(g). While PE
        # runs scores(g+1), vector builds esel(g) from the already-finished
        # e(g), so pooling(g)'s matmuls never wait tile-by-tile on vector.
        #
        # DMA trigger queues are segregated so that a trigger parked on DMA
        # backpressure never head-of-line-blocks latency-critical compute:
        # sync carries the xt loads, gpsimd the xn loads (prefetched three
        # groups ahead of use) + scatter + normalize loads, scalar stays
        # DMA-free for tanh/exp/flush-copy, vector stays esel-only.
        e_tiles: dict[int, object] = {}
        xn_tiles: dict[int, object] = {}

        def emit_scores(g):
            # s_nat[p, c] = score(row 2048g + 128c + p); two PSUM tiles
            # (banks) so each half's exp can fire without waiting for (or
            # bank-serializing against) the other half's matmuls
            xt0 = xT_pool.tile([128, G_ROWS], BF16, tag="xt0")
            xt1 = xT_pool.tile([128, G_ROWS], BF16, tag="xt1")
            if g == 0:
                for q in range(4):
                    sl4 = slice(512 * q, 512 * (q + 1))
                    nc.sync.dma_start(xt0[:, sl4], xT[0:128, sl4])
                    nc.sync.dma_start(xt1[:, sl4], xT[128:256, sl4])
            else:
                nc.sync.dma_start(xt0[:], xT[0:128, g * G_ROWS : (g + 1) * G_ROWS])
                nc.sync.dma_start(xt1[:], xT[128:256, g * G_ROWS : (g + 1) * G_ROWS])
            e_t = e_pool.tile([128, TILES_PER_G], F32, tag="e")
            for half in range(2):
                snat = s_psum.tile([128, 8], F32, tag=f"snat{half}")
                for ii in range(SUB_PER_G // 2):
                    i = 2 * half + ii
                    sl = slice(512 * i, 512 * (i + 1))
                    u = u_psum.tile([H, 512], F32, tag="u")
                    nc.tensor.matmul(u[:], w1c_t[:, 0:H], xt0[:, sl], start=True, stop=False)
                    nc.tensor.matmul(u[:], w1c_t[:, H : 2 * H], xt1[:, sl], start=False, stop=True)
                    h_t = h_pool.tile([H, 512], BF16, tag="h")
                    nc.scalar.activation(h_t[:], u[:], mybir.ActivationFunctionType.Tanh, bias=b1_t[:, 0:1])
                    for j in range(4):
                        lc = 4 * ii + j
                        nc.tensor.matmul(
                            snat[:, lc : lc + 1],
                            h_t[:, 128 * j : 128 * (j + 1)],
                            w2_t[:],
                            start=(lc == 0),
                            stop=(lc == 7),
                            skip_group_check=True,
                        )
                nc.scalar.activation(
                    e_t[:, 8 * half : 8 * (half + 1)],
                    snat[:],
                    mybir.ActivationFunctionType.Exp,
                    bias=float(b2_val),
                )
            e_tiles[g] = e_t

        def emit_xn_load(g):
            xn = xnat_pool.tile([128, TILES_PER_G * (D + 2)], xdt, tag="xn")
            t0 = g * TILES_PER_G
            xn3 = xn[:].rearrange("p (t d) -> p t d", d=D + 2)
            if g < 2:
                # warmup: 4-tile slabs so pooling(0) starts on the first
                # quarter instead of the whole 1MB group load
                for q in range(4):
                    nc.gpsimd.dma_start(
                        xn3[:, 4 * q : 4 * (q + 1), :],
                        x_nat[:, t0 + 4 * q : t0 + 4 * (q + 1), :],
                    )
            else:
                nc.gpsimd.dma_start(xn3, x_nat[:, t0 : t0 + TILES_PER_G, :])
            xn_tiles[g] = xn

        # pooling accumulates [64 segs, 256 pooled + 1 sum] in ONE PSUM tile
        # across a superblock of SUP_GROUPS groups (their combined segment
        # span fits the 64-wide window; host asserts), so the flush copy +
        # scatter-accumulate run once per superblock instead of per group —
        # 3x fewer gpsimd scatter instructions and DMA-semaphore recycles.
        sup_state: dict[str, object] = {"pooled": None}

        def emit_pooling(g):
            e_t = e_tiles.pop(g)
            xn = xn_tiles.pop(g)
            sup = g // SUP_GROUPS
            g_in_sup = g % SUP_GROUPS
            is_last_in_sup = g_in_sup == SUP_GROUPS - 1 or g == n_groups - 1
            if g_in_sup == 0:
                pooled_new = p_psum.tile([128, 257], F32, tag="pooled")
                sup_state["pooled"] = pooled_new
            pooled = sup_state["pooled"]
            for c in range(TILES_PER_G):
                t_abs = g * TILES_PER_G + c
                esel = esel_pool.tile([128, W_SEG], edt, tag="esel")
                nc.vector.tensor_scalar(
                    esel[:],
                    iota_t[:],
                    bl_t[:, t_abs : t_abs + 1],
                    e_t[:, c : c + 1],
                    mybir.AluOpType.is_equal,
                    mybir.AluOpType.mult,
                )
                rhs = xn[:, c * (D + 2) : c * (D + 2) + 257]
                if xdt == F32:
                    rhs = rhs.bitcast(F32R)
                nc.tensor.matmul(
                    pooled[0:W_SEG, 0:257], esel[:], rhs,
                    start=(g_in_sup == 0 and c == 0),
                    stop=(is_last_in_sup and c == TILES_PER_G - 1),
                    skip_group_check=True,
                )
            if is_last_in_sup:
                # flush: psum -> sbuf (scalar engine) -> scatter-accumulate
                fl = flush_pool.tile([W_SEG, 257], F32, tag="fl")
                nc.scalar.activation(fl[:], pooled[0:W_SEG, :], mybir.ActivationFunctionType.Copy)
                nc.gpsimd.indirect_dma_start(
                    scratch[:, :],
                    IndirectOffsetOnAxis(ap=sidx_t[:, sup : sup + 1], axis=0),
                    fl[:],
                    None,
                    compute_op=mybir.AluOpType.add,
                )

        emit_scores(0)
        for gp in range(min(3, n_groups)):
            emit_xn_load(gp)
        for g in range(n_groups):
            if g + 1 < n_groups:
                emit_scores(g + 1)
            if g + 3 < n_groups:
                emit_xn_load(g + 3)
            emit_pooling(g)

        # ---- final normalize, fully pipelined at the end: all block loads
        # issue together (the first carries the scatter fan-in wait), then
        # the vector divides, then the stores. Mid-stream normalize was tried
        # and reverted: its scatter-fan-in waits head-of-line-blocked the
        # esel stream (vector) or the xt prefetch (sync) for ~7us a block.
        # loads on sync: it is idle by now (xt loads done), so the scatter
        # fan-in wait parks nothing, and issuing there overlaps the gpsimd
        # queue still processing the last scatter instruction. Only 11
        # scatter sems exist now (superblocks), so the fan-in fits the sync
        # DGE wait slots that 31 per-group scatters used to overflow.
        n_blocks = SEGS_PER_CORE // 128  # 4 real blocks; pad rows never read
        ft_all = fin_pool.tile([128, n_blocks * 257], F32, tag="ft")
        nc.sync.dma_start(
            ft_all[:].rearrange("p (b c) -> p b c", c=257),
            scratch[0 : 128 * n_blocks, :].rearrange("(b p) c -> p b c", p=128),
        )
        ot_all = fin_pool.tile([128, n_blocks * D], F32, tag="ot")
        for b in range(n_blocks):
            rec = fin_pool.tile([128, 1], F32, tag="rec")
            nc.vector.reciprocal(rec[:], ft_all[:, b * 257 + D : b * 257 + D + 1])
            nc.vector.tensor_scalar(
                ot_all[:, b * D : (b + 1) * D],
                ft_all[:, b * 257 : b * 257 + D],
                rec[:, 0:1],
                None,
                mybir.AluOpType.mult,
            )
        nc.sync.dma_start(
            out[0 : 128 * n_blocks, :].rearrange("(b p) c -> p b c", p=128),
            ot_all[:].rearrange("p (b c) -> p b c", c=D),
        )

    return nc


def kernel(x, batch, W1, b1, W2, b2):
    x = np.asarray(x, dtype=np.float32)
    batch = np.asarray(batch)
    W1 = np.asarray(W1, dtype=np.float32)
    b1 = np.asarray(b1, dtype=np.float32)
    W2 = np.asarray(W2, dtype=np.float32)
    b2 = np.asarray(b2, dtype=np.float32)
    n, d = x.shape
    assert d == D

    bounds = np.searchsorted(batch, np.arange(NUM_SEGMENTS + 1))
    core_starts = [int(bounds[SEGS_PER_CORE * m]) for m in range(N_CORES + 1)]
    rows_per_core = [core_starts[m + 1] - core_starts[m] for m in range(N_CORES)]
    n_groups = max(1, int(np.ceil(max(rows_per_core) / G_ROWS)))
    r_pad = n_groups * G_ROWS
    n_tiles = n_groups * TILES_PER_G

    xdt = np.float32 if X_POOL_DTYPE == "f32" else ml_dtypes.bfloat16

    # shared constant inputs
    w1c = np.ascontiguousarray(
        W1.reshape(2, 128, H).astype(ml_dtypes.bfloat16)
    )
    w2col = np.ascontiguousarray(W2.reshape(H, 1).astype(ml_dtypes.bfloat16))
    b1col = np.ascontiguousarray(b1.reshape(H, 1))
    iota64 = np.broadcast_to(np.arange(W_SEG), (128, W_SEG)).astype(ml_dtypes.bfloat16)
    b2_val = float(b2.reshape(-1)[0])

    in_maps = []
    core_s0s = []
    for m in range(N_CORES):
        rs, re = core_starts[m], core_starts[m + 1]
        rows = re - rs
        xm = x[rs:re]
        x_flat = np.zeros((r_pad, D + 2), dtype=xdt)
        x_flat[:rows, :D] = xm.astype(xdt)
        x_flat[:rows, D] = xdt(1.0)
        # partition-major: x_nat[p, t, :] = x_flat[128t + p, :]
        x_nat = np.ascontiguousarray(
            x_flat.reshape(n_tiles, 128, D + 2).transpose(1, 0, 2)
        )
        xT = np.zeros((D, r_pad), dtype=ml_dtypes.bfloat16)
        xT[:, :rows] = xm.T.astype(ml_dtypes.bfloat16)

        seg_local = (batch[rs:re] - SEGS_PER_CORE * m).astype(np.int64)
        assert seg_local.min() >= 0 and seg_local.max() < SEGS_PER_CORE

        bl = np.full((128, n_tiles), PAD_BL, dtype=np.float32)
        n_sup = (n_groups + SUP_GROUPS - 1) // SUP_GROUPS
        sidx = np.empty((W_SEG, n_sup), dtype=np.int32)
        s0s = np.empty(n_sup, dtype=np.int64)
        for sup in range(n_sup):
            lo = sup * SUP_GROUPS * G_ROWS
            hi = min((sup + 1) * SUP_GROUPS * G_ROWS, rows)
            if lo >= rows:
                s0 = SEGS_PER_CORE  # pad region
            else:
                s0 = int(seg_local[lo])
                span = int(seg_local[hi - 1]) - s0
                assert span < W_SEG, f"superblock seg span {span} >= {W_SEG}"
                rr = np.arange(lo, hi)
                p = rr % 128
                c = (rr % G_ROWS) // 128
                gg = rr // G_ROWS
                bl[p, gg * TILES_PER_G + c] = (seg_local[lo:hi] - s0).astype(np.float32)
            sidx[:, sup] = s0 + np.arange(W_SEG)
            s0s[sup] = s0
        core_s0s.append(s0s)
        in_maps.append(
            {
                "x_nat": x_nat,
                "xT": xT,
                "w1c": w1c,
                "w2col": w2col,
                "b1col": b1col,
                "iota64": iota64,
                "bl_all": bl,
                "seg_idx": sidx,
            }
        )

    nc = build_nc(n_groups, b2_val)
    if not nc.is_finalized():
        nc.finalize()
    trace = os.environ.get("KERNEL_TRACE", "0") == "1"
    kw = {}
    if trace:
        kw = dict(trace=True, tmpdir=os.environ.get("KERNEL_TRACE_DIR") or None)
    res = run_bass_kernel_spmd(nc, in_maps, core_ids=list(range(N_CORES)), **kw)
    global LAST_EXEC_NS
    LAST_EXEC_NS = res.exec_time_ns
    if trace:
        print(
            f"exec_time_ns={res.exec_time_ns} mean={res.mean_exec_time_ns} "
            f"max_core={res.max_exec_time_core_id}",
            flush=True,
        )
    outs = res.results

    full = np.empty((NUM_SEGMENTS, D), dtype=np.float32)
    for m in range(N_CORES):
        full[SEGS_PER_CORE * m : SEGS_PER_CORE * (m + 1)] = outs[m]["out"][
            :SEGS_PER_CORE
        ]
    return full

